# revision 1
# baseline (speedup 1.0000x reference)
"""Chunked-prefill paged attention kernel for Trainium2 (Bass/Tile), 8 cores.

Sharding: tensor-parallel over heads. Core i handles q heads 4i..4i+3 and
kv head i. The paged-cache scatter/gather (pure data movement, index-driven)
is resolved on the host; each core runs dense attention over the gathered
[ctx | chunk] keys/values for its kv head.

Per-core layout ("transposed scores"): q and k arrive pre-transposed from the
host ([d, seq] / [d, L]), so the PE runs only the three matmul passes:
  scoresT[l, q] = kT_tile (stationary) x qT (moving)     -> PSUM
  exp on the scalar engine (PSUM -> SBUF, fp32r)
  oT[d, q]     += v_tile (stationary) x expT (moving)    -> PSUM
  sums[1, q]   += ones   (stationary) x expT (moving)    -> PSUM
The unnormalized oT and the softmax denominators are DMA'd out; the host
does the final divide and the [d, q] -> [q, d] transpose (cheap numpy).

All matmuls run in float32r (full 1 cycle/row stream rate at fp32 storage
precision for the accumulate; operands rounded by their producer ops).
"""

import numpy as np

import concourse.bacc as bacc
import concourse.bass as bass
import concourse.mybir as mybir
import concourse.tile as tile
from concourse.bass_utils import run_bass_kernel_spmd

NH, NKVH, HD = 32, 8, 128
SCALE = 0.08838834764831845  # 1/sqrt(128)
SEQ, CTX = 1024, 3072
L = CTX + SEQ  # 4096
NDEV = 8
HPD = NH // NDEV  # q heads per device
QCH = 512  # q columns per moving block (psum bank width in f32)
NQC = SEQ // QCH  # q chunks
NT = L // 128  # 32 l-tiles total
NT_CTX = CTX // 128  # 24 context l-tiles
NEG = -1.0e30

F32 = mybir.dt.float32
F32R = mybir.dt.float32r
BF16 = mybir.dt.bfloat16

# dtype for all big matmul operands; fp32r streams at 1 cycle/row for
# moving dims >= 256 and keeps near-fp32 accuracy.
MM_DT = F32R

_CACHE = {}


def _build():
    nc = bacc.Bacc("TRN2", target_bir_lowering=False, debug=False)

    qdT = nc.dram_tensor("qdT", [HPD * HD, SEQ], F32, kind="ExternalInput")
    kdT = nc.dram_tensor("kdT", [HD, L], F32, kind="ExternalInput")
    vd = nc.dram_tensor("vd", [L, HD], F32, kind="ExternalInput")
    tri = nc.dram_tensor("tri", [128, 128], F32, kind="ExternalInput")
    od = nc.dram_tensor("od", [HPD * HD, SEQ], F32, kind="ExternalOutput")
    sums_out = nc.dram_tensor("sums", [HPD, SEQ], F32, kind="ExternalOutput")

    with tile.TileContext(nc) as tc:
        with (
            tc.tile_pool(name="big", bufs=1) as big,
            tc.tile_pool(name="small", bufs=1) as small,
            tc.tile_pool(name="expp", bufs=4) as expp,
            tc.tile_pool(name="osb", bufs=2) as osb,
            tc.tile_pool(name="scps", bufs=2, space="PSUM") as scps,
            tc.tile_pool(name="accps", bufs=2, space="PSUM") as accps,
            tc.tile_pool(name="sumps", bufs=2, space="PSUM") as sumps,
        ):
            # ---- constants ----
            tri_sb = small.tile([128, 128], F32, tag="tri")
            nc.scalar.dma_start(out=tri_sb, in_=tri[:, :])
            ones_f = small.tile([128, 1], F32, tag="ones_f")
            nc.vector.memset(ones_f, 1.0)
            ones_sb = small.tile([128, 1], MM_DT, tag="ones")
            nc.vector.tensor_copy(out=ones_sb, in_=ones_f)

            # ---- loads + rounding casts (no PE work in prep) ----
            # separate tiles per chunk/head keep dependencies fine-grained.
            # k/q loads go on the SP HWDGE ring; v/tri on the ACT ring so the
            # first QK^T inputs are not queued behind 2MB of v.
            NKC = 4  # kT chunks (8 l-tiles each)
            kT_f = [big.tile([128, L // NKC], F32, name=f"kT_f{i}", tag=f"kT_f{i}") for i in range(NKC)]
            kT_c = [big.tile([128, L // NKC], MM_DT, name=f"kT{i}", tag=f"kT{i}") for i in range(NKC)]
            qT_f = [big.tile([128, SEQ], F32, name=f"qT_f{h}", tag=f"qT_f{h}") for h in range(HPD)]
            qT_h = [big.tile([128, SEQ], MM_DT, name=f"qT{h}", tag=f"qT{h}") for h in range(HPD)]
            v_f = [big.tile([128, NT // 4, HD], F32, name=f"v_f{i}", tag=f"v_f{i}") for i in range(4)]
            v_c = [big.tile([128, NT // 4, HD], MM_DT, name=f"v{i}", tag=f"v{i}") for i in range(4)]
            vdr = vd.rearrange("(t p) d -> p t d", p=128)

            # DMA emission order tracks first-consumer order; k/q interleave
            # on the SP ring, v (+tri) on the ACT ring.
            # chunk l-tiles (kT/v chunk 3) are consumed early (the masked
            # diagonal pairs are interleaved first), so load/cast them
            # right after chunk 0.
            for i in (0, 3, 1, 2):
                sl = slice(i * (L // NKC), (i + 1) * (L // NKC))
                nc.sync.dma_start(out=kT_f[i], in_=kdT[:, sl])
                if i == 0:
                    nc.sync.dma_start(out=qT_f[0], in_=qdT[0:128, :])
            for h in range(1, HPD):
                nc.sync.dma_start(
                    out=qT_f[h], in_=qdT[h * 128 : (h + 1) * 128, :]
                )
            for i in (0, 3, 1, 2):
                sl = slice(i * (NT // 4), (i + 1) * (NT // 4))
                nc.scalar.dma_start(out=v_f[i], in_=vdr[:, sl, :])

            # rounding casts on DVE, ordered for earliest consumer first
            for i, ap_pair in enumerate(
                [
                    (kT_c[0], kT_f[0]),
                    (qT_h[0], qT_f[0]),
                    (v_c[0], v_f[0]),
                    (kT_c[3], kT_f[3]),
                    (v_c[3], v_f[3]),
                    (kT_c[1], kT_f[1]),
                    (v_c[1], v_f[1]),
                    (kT_c[2], kT_f[2]),
                    (v_c[2], v_f[2]),
                    (qT_h[1], qT_f[1]),
                    (qT_h[2], qT_f[2]),
                    (qT_h[3], qT_f[3]),
                ]
            ):
                nc.vector.tensor_copy(out=ap_pair[0], in_=ap_pair[1])

            def kT_at(lt):
                return kT_c[lt // 8][:, (lt % 8) * 128 : (lt % 8 + 1) * 128]

            def v_at(lt):
                return v_c[lt // 8][:, lt % 8, :]

            # ---- main attention: one flat software pipeline over all
            # (head, q-chunk, l-tile-pair) tasks, so the PE never drains at
            # group boundaries: QK^T of task p+1 is emitted before PV of
            # task p.
            tasks = []  # (h, c, [lt, lt], first, last)
            for h in range(HPD):
                for c in range(NQC):
                    # pair each (narrow, masked) chunk tile with a full-width
                    # context tile early in the group, so the mask DVE/ACT
                    # chain always has enough PE work to hide behind;
                    # accumulation order is commutative.
                    n_chunk = 4 * c + 4
                    chunk = [NT_CTX + j for j in range(n_chunk)]
                    ctx = list(range(NT_CTX))
                    prs = [[ctx[j], chunk[j]] for j in range(n_chunk)]
                    rest = ctx[n_chunk:]
                    prs += [rest[i : i + 2] for i in range(0, len(rest), 2)]
                    for pi, pr in enumerate(prs):
                        tasks.append((h, c, pr, pi == 0, pi == len(prs) - 1))

            group_psum = {}  # (h, c) -> (acc, sums)
            ex_tiles = [None] * len(tasks)

            def start_col(lt, c):
                """first computed q column for this l-tile (left of it the
                row block is fully masked); fp32r wants moving dims >= 256
                so clamp the start."""
                if lt < NT_CTX:
                    return 0
                b = lt - NT_CTX - 4 * c
                return min(max(b, 0) * 128, QCH - 256)

            def emit_qkt(p):
                h, c, pr, _, _ = tasks[p]
                qmv = qT_h[h][:, c * QCH : (c + 1) * QCH]
                sc = scps.tile([128, 2, QCH], F32, tag="sc")
                ex = expp.tile([128, 2, QCH], MM_DT, tag="ex")
                ex_tiles[p] = ex
                for s, lt in enumerate(pr):
                    st = start_col(lt, c)
                    nc.tensor.matmul(
                        sc[:, s, st:],
                        kT_at(lt),
                        qmv[:, st:],
                        start=True,
                        stop=True,
                    )
                    b = lt - NT_CTX - 4 * c
                    mask_end = max(st, 128 * b if lt >= NT_CTX else 0)
                    if mask_end > 0:
                        # everything left of the diagonal block is fully
                        # masked: force exp() to exactly zero (covers both
                        # never-computed psum garbage and computed-but-
                        # masked blocks)
                        nc.vector.memset(sc[:, s, 0:mask_end], NEG)
                    if lt >= NT_CTX and 0 <= b <= 3:
                        nc.vector.tensor_add(
                            out=sc[:, s, b * 128 : (b + 1) * 128],
                            in0=sc[:, s, b * 128 : (b + 1) * 128],
                            in1=tri_sb,
                        )
                nc.scalar.activation(
                    out=ex,
                    in_=sc,
                    func=mybir.ActivationFunctionType.Exp,
                    scale=SCALE,
                )

            def emit_pv(p):
                h, c, pr, first, last = tasks[p]
                if first:
                    group_psum[(h, c)] = (
                        accps.tile([128, QCH], F32, name="acc", tag="acc"),
                        sumps.tile([1, QCH], F32, name="sums", tag="sums"),
                    )
                acc, sums = group_psum[(h, c)]
                ex = ex_tiles[p]
                for s, lt in enumerate(pr):
                    st = start_col(lt, c)
                    is_first = first and s == 0
                    is_last = last and s == len(pr) - 1
                    nc.tensor.matmul(
                        acc[:, st:],
                        v_at(lt),
                        ex[:, s, st:],
                        start=is_first,
                        stop=is_last,
                    )
                    nc.tensor.matmul(
                        sums[:, st:],
                        ones_sb,
                        ex[:, s, st:],
                        start=is_first,
                        stop=is_last,
                    )
                if last:
                    # epilogue: ship unnormalized oT + denominators
                    oT_sb = osb.tile([128, QCH], F32, tag="oT_sb")
                    nc.vector.tensor_copy(out=oT_sb, in_=acc)
                    sums_sb = osb.tile([1, QCH], F32, tag="sums_sb")
                    nc.vector.tensor_copy(out=sums_sb, in_=sums)
                    nc.sync.dma_start(
                        out=od[
                            h * 128 : (h + 1) * 128, c * QCH : (c + 1) * QCH
                        ],
                        in_=oT_sb,
                    )
                    nc.sync.dma_start(
                        out=sums_out[h : h + 1, c * QCH : (c + 1) * QCH],
                        in_=sums_sb,
                    )

            for p in range(len(tasks) + 1):
                if p < len(tasks):
                    emit_qkt(p)
                if p >= 1:
                    emit_pv(p - 1)
    nc.compile()
    return nc


def _prep_host(q, k, v, k_cache, v_cache, slot_mapping, context_slots):
    """Resolve the paged-cache scatter+gather on the host.

    Equivalent to: cache.at[slot_mapping].set(new); gather cache[context_slots];
    concat with the new chunk.
    """
    kh = np.ascontiguousarray(k).reshape(SEQ, NKVH, HD)
    vh = np.ascontiguousarray(v).reshape(SEQ, NKVH, HD)
    sm = np.asarray(slot_mapping)
    cs = np.asarray(context_slots)

    k_ctx = np.asarray(k_cache)[cs].copy()
    v_ctx = np.asarray(v_cache)[cs].copy()
    # overwrite any context slot that the new chunk was scattered into
    order = np.argsort(sm, kind="stable")
    ss = sm[order]
    j = np.searchsorted(ss, cs)
    jc = np.minimum(j, len(ss) - 1)
    hit = ss[jc] == cs
    if hit.any():
        src = order[jc[hit]]
        k_ctx[hit] = kh[src]
        v_ctx[hit] = vh[src]

    k_all = np.concatenate([k_ctx, kh], axis=0)  # [L, NKVH, HD]
    v_all = np.concatenate([v_ctx, vh], axis=0)
    return k_all, v_all


# results of the last run (exec time etc), for the local test harness
last_results = None


def kernel(q, k, v, k_cache, v_cache, slot_mapping, context_slots):
    global last_results
    q = np.asarray(q, dtype=np.float32)
    k_all, v_all = _prep_host(
        q, np.asarray(k), np.asarray(v), k_cache, v_cache, slot_mapping, context_slots
    )

    if "nc" not in _CACHE:
        _CACHE["nc"] = _build()
    nc = _CACHE["nc"]

    tri = np.where(
        np.arange(128)[None, :] >= np.arange(128)[:, None], 0.0, NEG
    ).astype(np.float32)

    in_maps = []
    for d in range(NDEV):
        in_maps.append(
            {
                "qdT": np.ascontiguousarray(
                    q[:, d * HPD * HD : (d + 1) * HPD * HD].T
                ),
                "kdT": np.ascontiguousarray(k_all[:, d, :].T),
                "vd": np.ascontiguousarray(v_all[:, d, :]),
                "tri": tri,
            }
        )

    res = run_bass_kernel_spmd(nc, in_maps, core_ids=list(range(NDEV)))
    last_results = res

    out = np.empty((SEQ, NH * HD), dtype=np.float32)
    for d in range(NDEV):
        oT = res.results[d]["od"].reshape(HPD, HD, SEQ)
        sums = res.results[d]["sums"]  # [HPD, SEQ]
        o = oT / sums[:, None, :]  # [HPD, HD, SEQ]
        out[:, d * HPD * HD : (d + 1) * HPD * HD] = (
            o.transpose(2, 0, 1).reshape(SEQ, HPD * HD)
        )
    return out



# revision 11
# speedup vs baseline: 1.3650x; 1.3650x over previous
"""Chunked-prefill paged attention kernel for Trainium2 (Bass/Tile), 8 cores.

Sharding: tensor-parallel over heads. Core i handles q heads 4i..4i+3 and
kv head i. The paged-cache scatter/gather (index-driven data movement) is
resolved on the host; each core runs dense attention over the gathered
[ctx | chunk] keys/values for its kv head.

Per-core structure ("transposed scores"): loop over (q-chunk c, head-pair
hp); inner loop over 128-row l-tiles, software-pipelined one step so the
activation engine (the bottleneck at ~1.15 us per [128,2,512] exp) never
starves:
  - 2 QK^T matmuls (fp16, kv-head kT stationary shared by both heads,
    LDWEIGHTS fully hidden behind the streams) -> fp32 PSUM pair tile
    [128,2,512] (2 banks, double-buffered).
  - causal mask: DVE adds a NEG-triangle on the diagonal 128-block; QK/PV
    and the exp are exactly trimmed to the visible q-columns.
  - ONE activation exps both heads' scores -> fp16 ex tile in SBUF.
  - 2 PV matmuls (fp16) accumulate into per-head PSUM banks.
  - 2 col-tiled (tile_position) ones-matmuls run CONCURRENTLY on separate
    XBUSes, accumulating both heads' softmax denominators into rows
    {0,32} of ONE persistent PSUM bank across the whole pass - one
    512-col stream per tile instead of two.
PSUM: 4 (score pairs x2) + 2 (accumulators) + 1 (denominators) = 7 banks.
The unnormalized oT and denominators are DMA'd out; the host does the
final divide and [d, q] -> [q, d] transpose.
"""

import numpy as np

import concourse.bacc as bacc
import concourse.bass as bass
import concourse.mybir as mybir
import concourse.tile as tile
from concourse.bass_utils import run_bass_kernel_spmd

NH, NKVH, HD = 32, 8, 128
SCALE = 0.08838834764831845  # 1/sqrt(128)
SEQ, CTX = 1024, 3072
L = CTX + SEQ  # 4096
NDEV = 8
HPD = NH // NDEV  # q heads per device
QCH = 512  # q columns per chunk (psum bank width in f32)
NQC = SEQ // QCH
NT = L // 128  # 32 l-tiles
NT_CTX = CTX // 128  # 24 context l-tiles
NEG = -1.0e30

F32 = mybir.dt.float32
FP16 = mybir.dt.float16

_CACHE = {}


def _tiles_for_chunk(c):
    """(lt, st, diag) per l-tile: st = first visible q-col, diag = needs
    triangular mask at cols [st, st+128)."""
    out = [(lt, 0, False) for lt in range(NT_CTX)]
    for b in range(4 * (c + 1)):
        st = 128 * b - QCH * c
        out.append((NT_CTX + b, max(st, 0), st >= 0))
    return out


def _build():
    nc = bacc.Bacc("TRN2", target_bir_lowering=False, debug=False)

    qdT = nc.dram_tensor("qdT", [HPD * HD, SEQ], FP16, kind="ExternalInput")
    kdT = nc.dram_tensor("kdT", [HD, L], FP16, kind="ExternalInput")
    vd = nc.dram_tensor("vd", [L, HD], F32, kind="ExternalInput")
    tri = nc.dram_tensor("tri", [128, 128], F32, kind="ExternalInput")
    od = nc.dram_tensor("od", [HPD * HD, SEQ], F32, kind="ExternalOutput")
    sums_out = nc.dram_tensor("sums", [HPD, SEQ], F32, kind="ExternalOutput")

    with tile.TileContext(nc) as tc:
        with (
            tc.tile_pool(name="inp", bufs=1) as inp,
            tc.tile_pool(name="small", bufs=1) as small,
            tc.tile_pool(name="exq", bufs=3) as exq,
            tc.tile_pool(name="ssb", bufs=2) as ssb,
            tc.tile_pool(name="osb", bufs=4) as osb,
            tc.tile_pool(name="scp", bufs=2, space="PSUM") as scp,
            tc.tile_pool(name="accps", bufs=1, space="PSUM") as accps,
            tc.tile_pool(name="sumps", bufs=1, space="PSUM") as sumps,
        ):
            # ---- constants ----
            tri_sb = small.tile([128, 128], F32, tag="tri")
            nc.scalar.dma_start(out=tri_sb, in_=tri[:, :])
            ones_f = small.tile([128, 1], F32, tag="ones_f")
            nc.vector.memset(ones_f, 1.0)
            ones_h = small.tile([128, 1], FP16, tag="ones")
            nc.vector.tensor_copy(out=ones_h, in_=ones_f)

            # ---- input loads (fp16 k/q direct; v f32 -> fp16 cast) ----
            NKC = 4
            kT = [
                inp.tile([128, L // NKC], FP16, name=f"kT{i}", tag=f"kT{i}")
                for i in range(NKC)
            ]
            qT = [
                inp.tile([128, SEQ], FP16, name=f"qT{h}", tag=f"qT{h}")
                for h in range(HPD)
            ]
            v_f = [
                inp.tile([128, NT // NKC, HD], F32, name=f"v_f{i}", tag=f"v_f{i}")
                for i in range(NKC)
            ]
            v_h = [
                inp.tile([128, NT // NKC, HD], FP16, name=f"v{i}", tag=f"v{i}")
                for i in range(NKC)
            ]
            vdr = vd.rearrange("(t p) d -> p t d", p=128)

            nc.sync.dma_start(out=kT[0], in_=kdT[:, 0 : L // NKC])
            for h in range(2):
                nc.sync.dma_start(
                    out=qT[h], in_=qdT[h * 128 : (h + 1) * 128, :]
                )
            for i in range(1, NKC):
                sl = slice(i * (L // NKC), (i + 1) * (L // NKC))
                nc.sync.dma_start(out=kT[i], in_=kdT[:, sl])
            for h in range(2, HPD):
                nc.sync.dma_start(
                    out=qT[h], in_=qdT[h * 128 : (h + 1) * 128, :]
                )
            for i in range(NKC):
                sl = slice(i * (NT // NKC), (i + 1) * (NT // NKC))
                nc.scalar.dma_start(out=v_f[i], in_=vdr[:, sl, :])
                nc.vector.tensor_copy(out=v_h[i], in_=v_f[i])

            def kT_at(lt):
                return kT[lt // 8][:, (lt % 8) * 128 : (lt % 8 + 1) * 128]

            def v_at(lt):
                return v_h[lt // 8][:, lt % 8, :]

            # ---- main: 4 passes (q-chunk c x head-pair hp) ----
            for c in range(NQC):
                tiles = _tiles_for_chunk(c)
                last_i = len(tiles) - 1
                for hp in range(HPD // 2):
                    h0 = 2 * hp
                    acc = [
                        accps.tile([128, QCH], F32, name=f"acc{j}", tag=f"acc{j}")
                        for j in range(2)
                    ]
                    sums_ps = sumps.tile(
                        [33, QCH], F32, name="sums_ps", tag="sums_ps"
                    )
                    ex_tiles = [None] * len(tiles)

                    def emit_qk(i, lt, st, diag):
                        qsl = slice(c * QCH + st, (c + 1) * QCH)
                        pair = scp.tile(
                            [128, 2, QCH], F32, name="pair", tag="pair"
                        )
                        for j in range(2):
                            nc.tensor.matmul(
                                pair[:, j, st:],
                                kT_at(lt),
                                qT[h0 + j][:, qsl],
                                start=True,
                                stop=True,
                            )
                            if diag:
                                nc.vector.tensor_add(
                                    out=pair[:, j, st : st + 128],
                                    in0=pair[:, j, st : st + 128],
                                    in1=tri_sb,
                                )
                        exi = exq.tile(
                            [128, 2, QCH], FP16, name="exi", tag="ex"
                        )
                        nc.scalar.activation(
                            out=exi[:, :, st:],
                            in_=pair[:, :, st:],
                            func=mybir.ActivationFunctionType.Exp,
                            scale=SCALE,
                        )
                        ex_tiles[i] = exi

                    def emit_pv(i, lt, st, diag):
                        exi = ex_tiles[i]
                        for j in range(2):
                            nc.tensor.matmul(
                                sums_ps[32 * j : 32 * j + 1, st:],
                                ones_h,
                                exi[:, j, st:],
                                start=(i == 0),
                                stop=(i == last_i),
                                tile_position=(0, 32 * j),
                                skip_group_check=True,
                            )
                        for j in range(2):
                            nc.tensor.matmul(
                                acc[j][:, st:],
                                v_at(lt),
                                exi[:, j, st:],
                                start=(i == 0),
                                stop=(i == last_i),
                                skip_group_check=True,
                            )

                    for i, (lt, st, diag) in enumerate(tiles):
                        emit_qk(i, lt, st, diag)
                        if i:
                            emit_pv(i - 1, *tiles[i - 1])
                    emit_pv(last_i, *tiles[last_i])

                    # ---- drains ----
                    sums_sb = ssb.tile([33, QCH], F32, tag="sums_sb")
                    nc.vector.tensor_copy(out=sums_sb, in_=sums_ps)
                    for j in range(2):
                        nc.sync.dma_start(
                            out=sums_out[
                                h0 + j : h0 + j + 1, c * QCH : (c + 1) * QCH
                            ],
                            in_=sums_sb[32 * j : 32 * j + 1, :],
                        )
                        acc_sb = osb.tile([128, QCH], F32, tag="acc_sb")
                        nc.vector.tensor_copy(out=acc_sb, in_=acc[j])
                        nc.sync.dma_start(
                            out=od[
                                (h0 + j) * 128 : (h0 + j + 1) * 128,
                                c * QCH : (c + 1) * QCH,
                            ],
                            in_=acc_sb,
                        )
    nc.compile()
    return nc


def _prep_host(q, k, v, k_cache, v_cache, slot_mapping, context_slots):
    """Resolve the paged-cache scatter+gather on the host."""
    kh = np.ascontiguousarray(k).reshape(SEQ, NKVH, HD)
    vh = np.ascontiguousarray(v).reshape(SEQ, NKVH, HD)
    sm = np.asarray(slot_mapping)
    cs = np.asarray(context_slots)

    k_ctx = np.asarray(k_cache)[cs].copy()
    v_ctx = np.asarray(v_cache)[cs].copy()
    order = np.argsort(sm, kind="stable")
    ss = sm[order]
    j = np.searchsorted(ss, cs)
    jc = np.minimum(j, len(ss) - 1)
    hit = ss[jc] == cs
    if hit.any():
        src = order[jc[hit]]
        k_ctx[hit] = kh[src]
        v_ctx[hit] = vh[src]

    k_all = np.concatenate([k_ctx, kh], axis=0)  # [L, NKVH, HD]
    v_all = np.concatenate([v_ctx, vh], axis=0)
    return k_all, v_all


# results of the last run (exec time etc), for the local test harness
last_results = None


def kernel(q, k, v, k_cache, v_cache, slot_mapping, context_slots):
    global last_results
    q = np.asarray(q, dtype=np.float32)
    k_all, v_all = _prep_host(
        q, np.asarray(k), np.asarray(v), k_cache, v_cache,
        slot_mapping, context_slots,
    )

    if "nc" not in _CACHE:
        _CACHE["nc"] = _build()
    nc = _CACHE["nc"]

    tri = np.where(
        np.arange(128)[None, :] >= np.arange(128)[:, None], 0.0, NEG
    ).astype(np.float32)

    in_maps = []
    for d in range(NDEV):
        in_maps.append(
            {
                "qdT": np.ascontiguousarray(
                    q[:, d * HPD * HD : (d + 1) * HPD * HD].T
                ).astype(np.float16),
                "kdT": np.ascontiguousarray(k_all[:, d, :].T).astype(
                    np.float16
                ),
                "vd": np.ascontiguousarray(v_all[:, d, :]),
                "tri": tri,
            }
        )

    res = run_bass_kernel_spmd(nc, in_maps, core_ids=list(range(NDEV)))
    last_results = res

    out = np.empty((SEQ, NH * HD), dtype=np.float32)
    for d in range(NDEV):
        oT = res.results[d]["od"].reshape(HPD, HD, SEQ)
        sums = res.results[d]["sums"]  # [HPD, SEQ]
        o = oT / sums[:, None, :]
        out[:, d * HPD * HD : (d + 1) * HPD * HD] = (
            o.transpose(2, 0, 1).reshape(SEQ, HPD * HD)
        )
    return out


# revision 12
# speedup vs baseline: 1.3852x; 1.0148x over previous
"""Chunked-prefill paged attention kernel for Trainium2 (Bass/Tile), 8 cores.

Sharding: tensor-parallel over heads. Core i handles q heads 4i..4i+3 and
kv head i. The paged-cache scatter/gather (index-driven data movement) is
resolved on the host; each core runs dense attention over the gathered
[ctx | chunk] keys/values for its kv head.

Per-core structure ("transposed scores"): loop over (q-chunk c, head-pair
hp); inner loop over 128-row l-tiles, software-pipelined one step so the
activation engine (the bottleneck at ~1.15 us per [128,2,512] exp) never
starves:
  - 2 QK^T matmuls (fp16, kv-head kT stationary shared by both heads,
    LDWEIGHTS fully hidden behind the streams) -> fp32 PSUM pair tile
    [128,2,512] (2 banks, double-buffered).
  - causal mask: DVE adds a NEG-triangle on the diagonal 128-block; QK/PV
    and the exp are exactly trimmed to the visible q-columns.
  - ONE activation exps both heads' scores -> fp16 ex tile in SBUF.
  - 2 PV matmuls (fp16) accumulate into per-head PSUM banks.
  - 2 col-tiled (tile_position) ones-matmuls run CONCURRENTLY on separate
    XBUSes, accumulating both heads' softmax denominators into rows
    {0,32} of ONE persistent PSUM bank across the whole pass - one
    512-col stream per tile instead of two.
PSUM: 4 (score pairs x2) + 2 (accumulators) + 1 (denominators) = 7 banks.
The unnormalized oT and denominators are DMA'd out; the host does the
final divide and [d, q] -> [q, d] transpose.
"""

import numpy as np

import concourse.bacc as bacc
import concourse.bass as bass
import concourse.mybir as mybir
import concourse.tile as tile
from concourse.bass_utils import run_bass_kernel_spmd

NH, NKVH, HD = 32, 8, 128
SCALE = 0.08838834764831845  # 1/sqrt(128)
SEQ, CTX = 1024, 3072
L = CTX + SEQ  # 4096
NDEV = 8
HPD = NH // NDEV  # q heads per device
QCH = 512  # q columns per chunk (psum bank width in f32)
NQC = SEQ // QCH
NT = L // 128  # 32 l-tiles
NT_CTX = CTX // 128  # 24 context l-tiles
NEG = -1.0e30

F32 = mybir.dt.float32
FP16 = mybir.dt.float16

_CACHE = {}


def _tiles_for_chunk(c):
    """(lt, st, diag) per l-tile: st = first visible q-col, diag = needs
    triangular mask at cols [st, st+128)."""
    out = [(lt, 0, False) for lt in range(NT_CTX)]
    for b in range(4 * (c + 1)):
        st = 128 * b - QCH * c
        out.append((NT_CTX + b, max(st, 0), st >= 0))
    return out


def _build():
    nc = bacc.Bacc("TRN2", target_bir_lowering=False, debug=False)

    qdT = nc.dram_tensor("qdT", [HPD * HD, SEQ], FP16, kind="ExternalInput")
    kdT = nc.dram_tensor("kdT", [HD, L], FP16, kind="ExternalInput")
    vd = nc.dram_tensor("vd", [L, HD], F32, kind="ExternalInput")
    tri = nc.dram_tensor("tri", [128, 128], FP16, kind="ExternalInput")
    od = nc.dram_tensor("od", [HPD * HD, SEQ], F32, kind="ExternalOutput")
    sums_out = nc.dram_tensor(
        "sums", [2, HPD, SEQ], F32, kind="ExternalOutput"
    )

    with tile.TileContext(nc) as tc:
        with (
            tc.tile_pool(name="inp", bufs=1) as inp,
            tc.tile_pool(name="small", bufs=1) as small,
            tc.tile_pool(name="exq", bufs=3) as exq,
            tc.tile_pool(name="ssb", bufs=2) as ssb,
            tc.tile_pool(name="osb", bufs=4) as osb,
            tc.tile_pool(name="scp", bufs=2, space="PSUM") as scp,
            tc.tile_pool(name="accps", bufs=1, space="PSUM") as accps,
            tc.tile_pool(name="sumps", bufs=1, space="PSUM") as sumps,
        ):
            # ---- constants ----
            tri_sb = small.tile([128, 128], FP16, tag="tri")
            nc.scalar.dma_start(out=tri_sb, in_=tri[:, :])
            ones_f = small.tile([128, 1], F32, tag="ones_f")
            nc.vector.memset(ones_f, 1.0)
            ones_h = small.tile([128, 1], FP16, tag="ones")
            nc.vector.tensor_copy(out=ones_h, in_=ones_f)

            # ---- input loads (fp16 k/q direct; v f32 -> fp16 cast) ----
            NKC = 4
            kT = [
                inp.tile([128, L // NKC], FP16, name=f"kT{i}", tag=f"kT{i}")
                for i in range(NKC)
            ]
            qT = [
                inp.tile([128, SEQ], FP16, name=f"qT{h}", tag=f"qT{h}")
                for h in range(HPD)
            ]
            v_f = [
                inp.tile([128, NT // NKC, HD], F32, name=f"v_f{i}", tag=f"v_f{i}")
                for i in range(NKC)
            ]
            v_h = [
                inp.tile([128, NT // NKC, HD], FP16, name=f"v{i}", tag=f"v{i}")
                for i in range(NKC)
            ]
            vdr = vd.rearrange("(t p) d -> p t d", p=128)

            nc.sync.dma_start(out=kT[0], in_=kdT[:, 0 : L // NKC])
            for h in range(2):
                nc.sync.dma_start(
                    out=qT[h], in_=qdT[h * 128 : (h + 1) * 128, :]
                )
            for i in range(1, NKC):
                sl = slice(i * (L // NKC), (i + 1) * (L // NKC))
                nc.sync.dma_start(out=kT[i], in_=kdT[:, sl])
            for h in range(2, HPD):
                nc.sync.dma_start(
                    out=qT[h], in_=qdT[h * 128 : (h + 1) * 128, :]
                )
            for i in range(NKC):
                sl = slice(i * (NT // NKC), (i + 1) * (NT // NKC))
                nc.scalar.dma_start(out=v_f[i], in_=vdr[:, sl, :])
                nc.vector.tensor_copy(out=v_h[i], in_=v_f[i])

            def kT_at(lt):
                return kT[lt // 8][:, (lt % 8) * 128 : (lt % 8 + 1) * 128]

            def v_at(lt):
                return v_h[lt // 8][:, lt % 8, :]

            # ---- main: 4 passes (q-chunk c x head-pair hp) ----
            for c in range(NQC):
                tiles = _tiles_for_chunk(c)
                last_i = len(tiles) - 1
                for hp in range(HPD // 2):
                    h0 = 2 * hp
                    acc = [
                        accps.tile([128, QCH], F32, name=f"acc{j}", tag=f"acc{j}")
                        for j in range(2)
                    ]
                    sums_ps = sumps.tile(
                        [97, QCH], F32, name="sums_ps", tag="sums_ps"
                    )
                    ex_tiles = [None] * len(tiles)

                    def emit_qk(i, lt, st, diag):
                        qsl = slice(c * QCH + st, (c + 1) * QCH)
                        pair = scp.tile(
                            [128, 2, QCH], F32, name="pair", tag="pair"
                        )
                        for j in range(2):
                            nc.tensor.matmul(
                                pair[:, j, st:],
                                kT_at(lt),
                                qT[h0 + j][:, qsl],
                                start=True,
                                stop=True,
                            )
                        exi = exq.tile(
                            [128, 2, QCH], FP16, name="exi", tag="ex"
                        )
                        nc.scalar.activation(
                            out=exi[:, :, st:],
                            in_=pair[:, :, st:],
                            func=mybir.ActivationFunctionType.Exp,
                            scale=SCALE,
                        )
                        if diag:
                            for j in range(2):
                                nc.vector.tensor_mul(
                                    out=exi[:, j, st : st + 128],
                                    in0=exi[:, j, st : st + 128],
                                    in1=tri_sb,
                                )
                        ex_tiles[i] = exi

                    def emit_tail(ia, ib):
                        # 4-way col-tiled concurrent denominator burst for
                        # two iterations: row = 64*(parity) + 32*(head)
                        for i in (ia, ib):
                            lt, st, diag = tiles[i]
                            exi = ex_tiles[i]
                            for j in range(2):
                                r = 64 * (i % 2) + 32 * j
                                nc.tensor.matmul(
                                    sums_ps[r : r + 1, st:],
                                    ones_h,
                                    exi[:, j, st:],
                                    start=(i < 2),
                                    stop=(i >= last_i - 1),
                                    tile_position=(0, r),
                                    skip_group_check=True,
                                )
                        for i in (ia, ib):
                            lt, st, diag = tiles[i]
                            exi = ex_tiles[i]
                            for j in range(2):
                                nc.tensor.matmul(
                                    acc[j][:, st:],
                                    v_at(lt),
                                    exi[:, j, st:],
                                    start=(i == 0),
                                    stop=(i == last_i),
                                    skip_group_check=True,
                                )

                    for pi in range(0, len(tiles), 2):
                        emit_qk(pi, *tiles[pi])
                        emit_qk(pi + 1, *tiles[pi + 1])
                        if pi:
                            emit_tail(pi - 2, pi - 1)
                    emit_tail(last_i - 1, last_i)

                    # ---- drains ----
                    sums_sb = ssb.tile([97, QCH], F32, tag="sums_sb")
                    nc.vector.tensor_copy(out=sums_sb, in_=sums_ps)
                    for j in range(2):
                        for p in range(2):
                            nc.sync.dma_start(
                                out=sums_out[
                                    p,
                                    h0 + j : h0 + j + 1,
                                    c * QCH : (c + 1) * QCH,
                                ],
                                in_=sums_sb[
                                    64 * p + 32 * j : 64 * p + 32 * j + 1, :
                                ],
                            )
                        acc_sb = osb.tile([128, QCH], F32, tag="acc_sb")
                        nc.vector.tensor_copy(out=acc_sb, in_=acc[j])
                        nc.sync.dma_start(
                            out=od[
                                (h0 + j) * 128 : (h0 + j + 1) * 128,
                                c * QCH : (c + 1) * QCH,
                            ],
                            in_=acc_sb,
                        )
    nc.compile()
    return nc


def _prep_host(q, k, v, k_cache, v_cache, slot_mapping, context_slots):
    """Resolve the paged-cache scatter+gather on the host."""
    kh = np.ascontiguousarray(k).reshape(SEQ, NKVH, HD)
    vh = np.ascontiguousarray(v).reshape(SEQ, NKVH, HD)
    sm = np.asarray(slot_mapping)
    cs = np.asarray(context_slots)

    k_ctx = np.asarray(k_cache)[cs].copy()
    v_ctx = np.asarray(v_cache)[cs].copy()
    order = np.argsort(sm, kind="stable")
    ss = sm[order]
    j = np.searchsorted(ss, cs)
    jc = np.minimum(j, len(ss) - 1)
    hit = ss[jc] == cs
    if hit.any():
        src = order[jc[hit]]
        k_ctx[hit] = kh[src]
        v_ctx[hit] = vh[src]

    k_all = np.concatenate([k_ctx, kh], axis=0)  # [L, NKVH, HD]
    v_all = np.concatenate([v_ctx, vh], axis=0)
    return k_all, v_all


# results of the last run (exec time etc), for the local test harness
last_results = None


def kernel(q, k, v, k_cache, v_cache, slot_mapping, context_slots):
    global last_results
    q = np.asarray(q, dtype=np.float32)
    k_all, v_all = _prep_host(
        q, np.asarray(k), np.asarray(v), k_cache, v_cache,
        slot_mapping, context_slots,
    )

    if "nc" not in _CACHE:
        _CACHE["nc"] = _build()
    nc = _CACHE["nc"]

    tri = np.where(
        np.arange(128)[None, :] >= np.arange(128)[:, None], 1.0, 0.0
    ).astype(np.float16)

    in_maps = []
    for d in range(NDEV):
        in_maps.append(
            {
                "qdT": np.ascontiguousarray(
                    q[:, d * HPD * HD : (d + 1) * HPD * HD].T
                ).astype(np.float16),
                "kdT": np.ascontiguousarray(k_all[:, d, :].T).astype(
                    np.float16
                ),
                "vd": np.ascontiguousarray(v_all[:, d, :]),
                "tri": tri,
            }
        )

    res = run_bass_kernel_spmd(nc, in_maps, core_ids=list(range(NDEV)))
    last_results = res

    out = np.empty((SEQ, NH * HD), dtype=np.float32)
    for d in range(NDEV):
        oT = res.results[d]["od"].reshape(HPD, HD, SEQ)
        s2 = res.results[d]["sums"]  # [2, HPD, SEQ]
        sums = s2[0] + s2[1]
        o = oT / sums[:, None, :]
        out[:, d * HPD * HD : (d + 1) * HPD * HD] = (
            o.transpose(2, 0, 1).reshape(SEQ, HPD * HD)
        )
    return out


# revision 13
# speedup vs baseline: 1.5475x; 1.1171x over previous
"""Chunked-prefill paged attention kernel for Trainium2 (Bass/Tile), 8 cores.

Sharding: tensor-parallel over heads. Core i handles q heads 4i..4i+3 and
kv head i. The paged-cache scatter/gather (index-driven data movement) is
resolved on the host; each core runs dense attention over the gathered
[ctx | chunk] keys/values for its kv head.

Per-core structure ("transposed scores"): loop over (q-chunk c, head-pair
hp); inner loop over 128-row l-tiles, software-pipelined one step so the
activation engine (the bottleneck at ~1.15 us per [128,2,512] exp) never
starves:
  - 2 QK^T matmuls (fp16, kv-head kT stationary shared by both heads,
    LDWEIGHTS fully hidden behind the streams) -> fp32 PSUM pair tile
    [128,2,512] (2 banks, double-buffered).
  - causal mask: DVE adds a NEG-triangle on the diagonal 128-block; QK/PV
    and the exp are exactly trimmed to the visible q-columns.
  - ONE activation exps both heads' scores -> fp16 ex tile in SBUF.
  - 2 PV matmuls (fp16) accumulate into per-head PSUM banks.
  - 2 col-tiled (tile_position) ones-matmuls run CONCURRENTLY on separate
    XBUSes, accumulating both heads' softmax denominators into rows
    {0,32} of ONE persistent PSUM bank across the whole pass - one
    512-col stream per tile instead of two.
PSUM: 4 (score pairs x2) + 2 (accumulators) + 1 (denominators) = 7 banks.
The unnormalized oT and denominators are DMA'd out; the host does the
final divide and [d, q] -> [q, d] transpose.
"""

import numpy as np

import concourse.bacc as bacc
import concourse.bass as bass
import concourse.mybir as mybir
import concourse.tile as tile
from concourse.bass_utils import run_bass_kernel_spmd

NH, NKVH, HD = 32, 8, 128
SCALE = 0.08838834764831845  # 1/sqrt(128)
SEQ, CTX = 1024, 3072
L = CTX + SEQ  # 4096
NDEV = 8
HPD = NH // NDEV  # q heads per device
QCH = 512  # q columns per chunk (psum bank width in f32)
NQC = SEQ // QCH
NT = L // 128  # 32 l-tiles
NT_CTX = CTX // 128  # 24 context l-tiles
NEG = -1.0e30

F32 = mybir.dt.float32
FP16 = mybir.dt.float16

_CACHE = {}


def _tiles_for_chunk(c):
    """(lt, st, diag) per l-tile: st = first visible q-col, diag = needs
    triangular mask at cols [st, st+128)."""
    out = [(lt, 0, False) for lt in range(NT_CTX)]
    for b in range(4 * (c + 1)):
        st = 128 * b - QCH * c
        out.append((NT_CTX + b, max(st, 0), st >= 0))
    return out


def _build():
    nc = bacc.Bacc("TRN2", target_bir_lowering=False, debug=False)

    qdT = nc.dram_tensor("qdT", [HPD * HD, SEQ], FP16, kind="ExternalInput")
    kdT = nc.dram_tensor("kdT", [HD, L], FP16, kind="ExternalInput")
    vd = nc.dram_tensor("vd", [L, HD], F32, kind="ExternalInput")
    tri = nc.dram_tensor("tri", [128, 128], FP16, kind="ExternalInput")
    od = nc.dram_tensor("od", [HPD * HD, SEQ], F32, kind="ExternalOutput")
    sums_out = nc.dram_tensor(
        "sums", [2, HPD, SEQ], F32, kind="ExternalOutput"
    )

    with tile.TileContext(nc) as tc:
        with (
            tc.tile_pool(name="inp", bufs=1) as inp,
            tc.tile_pool(name="small", bufs=1) as small,
            tc.tile_pool(name="exq", bufs=6) as exq,
            tc.tile_pool(name="ssb", bufs=2) as ssb,
            tc.tile_pool(name="osb", bufs=4) as osb,
            tc.tile_pool(name="scp", bufs=2, space="PSUM") as scp,
            tc.tile_pool(name="accps", bufs=1, space="PSUM") as accps,
            tc.tile_pool(name="sumps", bufs=1, space="PSUM") as sumps,
        ):
            # ---- constants ----
            tri_sb = small.tile([128, 128], FP16, tag="tri")
            nc.scalar.dma_start(out=tri_sb, in_=tri[:, :])
            ones_f = small.tile([128, 1], F32, tag="ones_f")
            nc.vector.memset(ones_f, 1.0)
            ones_h = small.tile([128, 1], FP16, tag="ones")
            nc.vector.tensor_copy(out=ones_h, in_=ones_f)

            # ---- input loads (fp16 k/q direct; v f32 -> fp16 cast) ----
            NKC = 4
            kT = [
                inp.tile([128, L // NKC], FP16, name=f"kT{i}", tag=f"kT{i}")
                for i in range(NKC)
            ]
            qT = [
                inp.tile([128, SEQ], FP16, name=f"qT{h}", tag=f"qT{h}")
                for h in range(HPD)
            ]
            v_f = [
                inp.tile([128, NT // NKC, HD], F32, name=f"v_f{i}", tag=f"v_f{i}")
                for i in range(NKC)
            ]
            v_h = [
                inp.tile([128, NT // NKC, HD], FP16, name=f"v{i}", tag=f"v{i}")
                for i in range(NKC)
            ]
            vdr = vd.rearrange("(t p) d -> p t d", p=128)

            nc.sync.dma_start(out=kT[0], in_=kdT[:, 0 : L // NKC])
            for h in range(2):
                nc.sync.dma_start(
                    out=qT[h], in_=qdT[h * 128 : (h + 1) * 128, :]
                )
            for i in range(1, NKC):
                sl = slice(i * (L // NKC), (i + 1) * (L // NKC))
                nc.sync.dma_start(out=kT[i], in_=kdT[:, sl])
            for h in range(2, HPD):
                nc.sync.dma_start(
                    out=qT[h], in_=qdT[h * 128 : (h + 1) * 128, :]
                )
            for i in range(NKC):
                sl = slice(i * (NT // NKC), (i + 1) * (NT // NKC))
                nc.scalar.dma_start(out=v_f[i], in_=vdr[:, sl, :])
                nc.vector.tensor_copy(out=v_h[i], in_=v_f[i])

            def kT_at(lt):
                return kT[lt // 8][:, (lt % 8) * 128 : (lt % 8 + 1) * 128]

            def v_at(lt):
                return v_h[lt // 8][:, lt % 8, :]

            # ---- main: 4 passes (q-chunk c x head-pair hp) ----
            for c in range(NQC):
                tiles = _tiles_for_chunk(c)
                last_i = len(tiles) - 1
                for hp in range(HPD // 2):
                    h0 = 2 * hp
                    acc = [
                        accps.tile([128, QCH], F32, name=f"acc{j}", tag=f"acc{j}")
                        for j in range(2)
                    ]
                    sums_ps = sumps.tile(
                        [97, QCH], F32, name="sums_ps", tag="sums_ps"
                    )
                    ex_tiles = [None] * len(tiles)

                    def emit_qk(i, lt, st, diag):
                        qsl = slice(c * QCH + st, (c + 1) * QCH)
                        pair = scp.tile(
                            [128, 2, QCH], F32, name="pair", tag="pair"
                        )
                        for j in range(2):
                            nc.tensor.matmul(
                                pair[:, j, st:],
                                kT_at(lt),
                                qT[h0 + j][:, qsl],
                                start=True,
                                stop=True,
                            )
                        exi = exq.tile(
                            [128, 2, QCH], FP16, name="exi", tag="ex"
                        )
                        nc.scalar.activation(
                            out=exi[:, :, st:],
                            in_=pair[:, :, st:],
                            func=mybir.ActivationFunctionType.Exp,
                            scale=SCALE,
                        )
                        if diag:
                            for j in range(2):
                                nc.vector.tensor_mul(
                                    out=exi[:, j, st : st + 128],
                                    in0=exi[:, j, st : st + 128],
                                    in1=tri_sb,
                                )
                        ex_tiles[i] = exi

                    def emit_tail(ia, ib):
                        # 4-way col-tiled concurrent denominator burst for
                        # two iterations: row = 64*(parity) + 32*(head)
                        for i in (ia, ib):
                            lt, st, diag = tiles[i]
                            exi = ex_tiles[i]
                            for j in range(2):
                                r = 64 * (i % 2) + 32 * j
                                nc.tensor.matmul(
                                    sums_ps[r : r + 1, st:],
                                    ones_h,
                                    exi[:, j, st:],
                                    start=(i < 2),
                                    stop=(i >= last_i - 1),
                                    tile_position=(0, r),
                                    skip_group_check=True,
                                )
                        for i in (ia, ib):
                            lt, st, diag = tiles[i]
                            exi = ex_tiles[i]
                            for j in range(2):
                                nc.tensor.matmul(
                                    acc[j][:, st:],
                                    v_at(lt),
                                    exi[:, j, st:],
                                    start=(i == 0),
                                    stop=(i == last_i),
                                    skip_group_check=True,
                                )

                    n = len(tiles)
                    for pi in range(0, n, 2):
                        emit_qk(pi, *tiles[pi])
                        emit_qk(pi + 1, *tiles[pi + 1])
                        if pi >= 4:
                            emit_tail(pi - 4, pi - 3)
                    emit_tail(n - 4, n - 3)
                    emit_tail(n - 2, n - 1)

                    # ---- drains ----
                    sums_sb = ssb.tile([97, QCH], F32, tag="sums_sb")
                    nc.vector.tensor_copy(out=sums_sb, in_=sums_ps)
                    for j in range(2):
                        for p in range(2):
                            nc.sync.dma_start(
                                out=sums_out[
                                    p,
                                    h0 + j : h0 + j + 1,
                                    c * QCH : (c + 1) * QCH,
                                ],
                                in_=sums_sb[
                                    64 * p + 32 * j : 64 * p + 32 * j + 1, :
                                ],
                            )
                        acc_sb = osb.tile([128, QCH], F32, tag="acc_sb")
                        nc.vector.tensor_copy(out=acc_sb, in_=acc[j])
                        nc.sync.dma_start(
                            out=od[
                                (h0 + j) * 128 : (h0 + j + 1) * 128,
                                c * QCH : (c + 1) * QCH,
                            ],
                            in_=acc_sb,
                        )
    nc.compile()
    return nc


def _prep_host(q, k, v, k_cache, v_cache, slot_mapping, context_slots):
    """Resolve the paged-cache scatter+gather on the host."""
    kh = np.ascontiguousarray(k).reshape(SEQ, NKVH, HD)
    vh = np.ascontiguousarray(v).reshape(SEQ, NKVH, HD)
    sm = np.asarray(slot_mapping)
    cs = np.asarray(context_slots)

    k_ctx = np.asarray(k_cache)[cs].copy()
    v_ctx = np.asarray(v_cache)[cs].copy()
    order = np.argsort(sm, kind="stable")
    ss = sm[order]
    j = np.searchsorted(ss, cs)
    jc = np.minimum(j, len(ss) - 1)
    hit = ss[jc] == cs
    if hit.any():
        src = order[jc[hit]]
        k_ctx[hit] = kh[src]
        v_ctx[hit] = vh[src]

    k_all = np.concatenate([k_ctx, kh], axis=0)  # [L, NKVH, HD]
    v_all = np.concatenate([v_ctx, vh], axis=0)
    return k_all, v_all


# results of the last run (exec time etc), for the local test harness
last_results = None


def kernel(q, k, v, k_cache, v_cache, slot_mapping, context_slots):
    global last_results
    q = np.asarray(q, dtype=np.float32)
    k_all, v_all = _prep_host(
        q, np.asarray(k), np.asarray(v), k_cache, v_cache,
        slot_mapping, context_slots,
    )

    if "nc" not in _CACHE:
        _CACHE["nc"] = _build()
    nc = _CACHE["nc"]

    tri = np.where(
        np.arange(128)[None, :] >= np.arange(128)[:, None], 1.0, 0.0
    ).astype(np.float16)

    in_maps = []
    for d in range(NDEV):
        in_maps.append(
            {
                "qdT": np.ascontiguousarray(
                    q[:, d * HPD * HD : (d + 1) * HPD * HD].T
                ).astype(np.float16),
                "kdT": np.ascontiguousarray(k_all[:, d, :].T).astype(
                    np.float16
                ),
                "vd": np.ascontiguousarray(v_all[:, d, :]),
                "tri": tri,
            }
        )

    res = run_bass_kernel_spmd(nc, in_maps, core_ids=list(range(NDEV)))
    last_results = res

    out = np.empty((SEQ, NH * HD), dtype=np.float32)
    for d in range(NDEV):
        oT = res.results[d]["od"].reshape(HPD, HD, SEQ)
        s2 = res.results[d]["sums"]  # [2, HPD, SEQ]
        sums = s2[0] + s2[1]
        o = oT / sums[:, None, :]
        out[:, d * HPD * HD : (d + 1) * HPD * HD] = (
            o.transpose(2, 0, 1).reshape(SEQ, HPD * HD)
        )
    return out
